# revision 2
# baseline (speedup 1.0000x reference)
"""Trainium2 Bass kernel for nn_BasicGRUBlock — transposed-state design.

  x = y + z; h1 = GRU0(x); h2 = GRU1(h1); out = y + h2 @ W_lin.T + b_lin

Sharding: data-parallel over batch across 8 cores (BL=8 sequences/core).

Key idea vs v1: keep the hidden state PERMANENTLY transposed, i.e. as
hT tiles [128(gate/h dim), 2(k-chunk), 8(batch)].  Gates then live on the
partition axis and batch on the (tiny) free axis, so every per-step
elementwise op costs fixed-overhead only (~16 free elems), sigmoid/tanh
run on [128,32]/[128,16] tiles, and the per-step PE transpose + copy of
the old design disappear.  The recurrent matmuls become
  ghT[128j:128j+128, b] += (W_hh[j,kc])^T-stationary @ hT[kc]  (N=8 moving)
with W chunks stationary and the tiny state moving.

Time is processed in super-groups of SUP=32 steps:
  P0: DMA y,z (time-major partition order), x = y+z, PE-transpose -> xT
  P1: gx0 bulk matmuls (N=256 moving) + PSUM->SBUF copies w/ bias fold
  P2: 32 recurrence steps of layer 0  -> h1T super tile
  P3: gx1 bulk matmuls from h1T + copies
  P4: 32 recurrence steps of layer 1 (for the PREVIOUS super; interleaved
      instruction-by-instruction with P2 so both chains overlap)
  P5: out = y + h2 @ W_lin^T + b_lin for the previous super, DMA out.

Supers alternate parity u=0/1; all cross-super tiles are persistent
per-parity (software ping-pong), so the For_i body holds two supers.
"""

import sys

sys.path.insert(0, "/opt/trn_rl_repo")

import numpy as np

import concourse.bass as bass
import concourse.bacc as bacc_mod
import concourse.mybir as mybir
from concourse.bass import ds
from concourse.tile import TileContext

B, T_FULL, I, H, G = 64, 4096, 64, 256, 768
NCORES = 8
BL = B // NCORES  # 8 sequences per core
SUP = 32          # time steps per super-group
NSUB = SUP // 8   # 8-step sub-blocks (for 64-wide transposes / out tiles)
NCH = 6           # gate chunks of 128 (r0 r1 z0 z1 n0 n1)
F32 = mybir.dt.float32
F32R = mybir.dt.float32r
BF16 = mybir.dt.bfloat16

SIG = mybir.ActivationFunctionType.Sigmoid
TANH = mybir.ActivationFunctionType.Tanh
IDENT = mybir.ActivationFunctionType.Identity
MULT = mybir.AluOpType.mult
ADD = mybir.AluOpType.add
SUB = mybir.AluOpType.subtract


def _r(ap):
    """View an fp32 AP as float32r for full-rate PE matmul."""
    return ap.bitcast(F32R)


def build_nc(T=T_FULL, debug=False):
    nc = bacc_mod.Bacc()

    y_d = nc.declare_dram_parameter("y", [BL, T, I], F32, isOutput=False)
    z_d = nc.declare_dram_parameter("z", [BL, T, I], F32, isOutput=False)
    w0T_d = nc.declare_dram_parameter("w0T", [I, NCH, 128], F32R, isOutput=False)
    whh0T_d = nc.declare_dram_parameter("whh0T", [128, 2, NCH, 128], BF16,
                                        isOutput=False)
    bcol0_d = nc.declare_dram_parameter("bcol0", [128, NCH], F32, isOutput=False)
    bhnB0_d = nc.declare_dram_parameter("bhnB0", [128, SUP, 2, BL], BF16,
                                        isOutput=False)
    w1T_d = nc.declare_dram_parameter("w1T", [128, 2, NCH, 128], BF16,
                                      isOutput=False)
    whh1T_d = nc.declare_dram_parameter("whh1T", [128, 2, NCH, 128], BF16,
                                        isOutput=False)
    bcol1_d = nc.declare_dram_parameter("bcol1", [128, NCH], F32, isOutput=False)
    bhnB1_d = nc.declare_dram_parameter("bhnB1", [128, SUP, 2, BL], BF16,
                                        isOutput=False)
    wlinT_d = nc.declare_dram_parameter("wlinT", [128, 2, I], BF16, isOutput=False)
    blin_d = nc.declare_dram_parameter("blin", [1, I], BF16, isOutput=False)
    eye64_d = nc.declare_dram_parameter("eye64", [64, 64], F32, isOutput=False)
    eye128_d = nc.declare_dram_parameter("eye128", [128, 128], BF16,
                                         isOutput=False)
    out_d = nc.declare_dram_parameter("out", [BL, T, I], F32, isOutput=True)
    dbg1_d = dbg2_d = None
    if debug:
        dbg1_d = nc.declare_dram_parameter("dbg1", [T // SUP, 128, 2, SUP, BL],
                                           BF16, isOutput=True)
        dbg2_d = nc.declare_dram_parameter("dbg2", [T // SUP, 128, 2, SUP, BL],
                                           BF16, isOutput=True)
        dbg3_d = nc.declare_dram_parameter("dbg3", [T // SUP, 128, SUP, 6, BL],
                                           BF16, isOutput=True)
        dbg4_d = nc.declare_dram_parameter("dbg4", [T // SUP, 128, SUP, 2, BL],
                                           F32, isOutput=True)

    assert T % (2 * SUP) == 0
    nsup = T // SUP

    with TileContext(nc) as tc:
        with (
            tc.tile_pool(name="wpool", bufs=1) as wpool,
            tc.tile_pool(name="gpool", bufs=2) as gpool,
            tc.tile_pool(name="ps_a0", bufs=2, space="PSUM") as ps_a0,
            tc.tile_pool(name="ps_a1", bufs=2, space="PSUM") as ps_a1,
            tc.tile_pool(name="ps_rz0", bufs=1, space="PSUM") as ps_rz0,
            tc.tile_pool(name="ps_rz1", bufs=1, space="PSUM") as ps_rz1,
            tc.tile_pool(name="ps_misc", bufs=2, space="PSUM") as ps_misc,
        ):
            # ---- persistent weights / constants ----
            w0T_t = wpool.tile([I, NCH, 128], F32R)
            whh0T_t = wpool.tile([128, 2, NCH, 128], BF16)
            bcol0_t = wpool.tile([128, NCH], F32)
            
            w1T_t = wpool.tile([128, 2, NCH, 128], BF16)
            whh1T_t = wpool.tile([128, 2, NCH, 128], BF16)
            bcol1_t = wpool.tile([128, NCH], F32)
            
            wlinT_t = wpool.tile([128, 2, I], BF16)
            blin_t = wpool.tile([1, I], BF16)
            eye64_t = wpool.tile([64, 64], F32)
            eye128_t = wpool.tile([128, 128], BF16)
            ones_f32 = wpool.tile([1, 64], F32)
            ones_bf = wpool.tile([1, 64], BF16)

            for t_, d_ in ((w0T_t, w0T_d), (whh0T_t, whh0T_d),
                           (bcol0_t, bcol0_d),
                           (w1T_t, w1T_d), (whh1T_t, whh1T_d),
                           (bcol1_t, bcol1_d),
                           (wlinT_t, wlinT_d), (blin_t, blin_d),
                           (eye64_t, eye64_d), (eye128_t, eye128_d)):
                nc.sync.dma_start(out=t_, in_=d_[:])
            nc.gpsimd.memset(ones_f32[:], 1.0)
            nc.vector.tensor_copy(ones_bf[:], ones_f32[:])

            # ---- per-parity persistent tiles (software ping-pong) ----
            y_t, z_t, x_t, xT_t = [], [], [], []
            gxA0, gxN0, gxA1, gxN1 = [], [], [], []
            h1T, h2T = [], []
            for u in (0, 1):
                y_t.append(wpool.tile([64, NSUB, I], F32, name=f"y{u}"))
                z_t.append(wpool.tile([64, NSUB, I], F32, name=f"z{u}"))
                x_t.append(wpool.tile([64, NSUB, I], F32, name=f"x{u}"))
                xT_t.append(wpool.tile([I, SUP * BL], F32R, name=f"xT{u}"))
                gxA0.append(wpool.tile([128, SUP, 6, BL], BF16, name=f"gxA0{u}"))
                gxN0.append(wpool.tile([128, SUP, 2, BL], F32, name=f"gxN0{u}"))
                gxA1.append(wpool.tile([128, SUP, 6, BL], BF16, name=f"gxA1{u}"))
                gxN1.append(wpool.tile([128, SUP, 2, BL], F32, name=f"gxN1{u}"))
                h1T.append(wpool.tile([128, 2, SUP, BL], BF16, name=f"h1T{u}"))
                h2T.append(wpool.tile([128, 2, SUP, BL], BF16, name=f"h2T{u}"))
            o_t = wpool.tile([64, NSUB, I], F32)

            zeros_f32 = wpool.tile([128, 2, SUP, BL], F32)
            nc.gpsimd.memset(zeros_f32[:], 0.0)
            nc.vector.tensor_copy(h1T[0][:], zeros_f32[:])
            nc.vector.tensor_copy(h1T[1][:], zeros_f32[:])
            nc.vector.tensor_copy(h2T[0][:], zeros_f32[:])
            nc.vector.tensor_copy(h2T[1][:], zeros_f32[:])
            for u in (0, 1):
                nc.sync.dma_start(out=gxA0[u][:, :, 4:6, :], in_=bhnB0_d[:])
                nc.sync.dma_start(out=gxA1[u][:, :, 4:6, :], in_=bhnB1_d[:])

            # ------------- building blocks -------------

            def p0_input(u, t0):
                """DMA y,z (time-major partitions), x=y+z, transpose -> xT."""
                for s in range(NSUB):
                    nc.sync.dma_start(
                        out=y_t[u][:, s, :],
                        in_=y_d[:, ds(t0 + 8 * s, 8), :].transpose([1, 0, 2]))
                    nc.sync.dma_start(
                        out=z_t[u][:, s, :],
                        in_=z_d[:, ds(t0 + 8 * s, 8), :].transpose([1, 0, 2]))
                nc.gpsimd.tensor_tensor(x_t[u], y_t[u], z_t[u], ADD)
                for s in range(NSUB):
                    tp = ps_misc.tile([64, 64], F32, tag="ps", name="tp")
                    nc.tensor.transpose(tp, x_t[u][:, s, :], eye64_t)
                    nc.scalar.copy(xT_t[u][:, ds(64 * s, 64)], tp)

            def gx_bulk(u, lhsT_t, moving, bcol_t, gxA, gxN, act_copy):
                """gx = W^T-stationary @ moving -> PSUM -> SBUF (+bias).

                moving: list of k-chunk APs [K, 256] (1 for L0, 2 for L1).
                """
                for j in range(NCH):
                    pg = ps_misc.tile([128, SUP, BL], F32, tag="ps", name="pg")
                    nk = len(moving)
                    for kc in range(nk):
                        lw = lhsT_t[:, kc, j, :] if nk > 1 else lhsT_t[:, j, :]
                        if lw.dtype == F32:
                            lw = _r(lw)
                        nc.tensor.matmul(pg, lw, moving[kc],
                                         start=(kc == 0), stop=(kc == nk - 1))
                    dst = gxA[u][:, :, j, :] if j < 4 else gxN[u][:, :, j - 4, :]
                    if act_copy:
                        nc.scalar.activation(dst, pg, IDENT,
                                             bias=bcol_t[:, j:j + 1])
                    else:
                        nc.vector.tensor_scalar(dst, pg, bcol_t[:, j:j + 1],
                                                None, ADD)

            def step_phases(k, hT_c, hT_p, gxA, gxN, whhT_t, psA_pool,
                            rz_pool, lt):
                """Return the per-engine phases of one transposed GRU step so
                two layers' steps can be interleaved stage-by-stage.
                psAB bank layout: [128, 6, BL] = r0 r1 z0 z1 | n0+bhn n1+bhn.
                h' = n*(1-z) + z*h  (u=z*h, v=1-z overlap the tanh)."""
                hsrc = hT_p[:, :, SUP - 1, :] if k == 0 else hT_c[:, :, k - 1, :]
                psAB = psA_pool.tile([128, 6, BL], F32, tag=f"a{lt}",
                                     name="psAB")
                st = {}

                def pe():
                    nc.tensor.matmul(psAB, eye128_t, gxA[:, k, :, :],
                                     start=True, stop=False)
                    for j in range(NCH):
                        for kc in range(2):
                            nc.tensor.matmul(psAB[:, j, :],
                                             whhT_t[:, kc, j, :],
                                             hsrc[:, kc, :], start=False,
                                             stop=(j == NCH - 1 and kc == 1))

                def sig():
                    st["rz"] = gpool.tile([128, 4, BL], F32, tag=f"rz{lt}",
                                          name="rz")
                    nc.scalar.activation(st["rz"], psAB[:, 0:4, :], SIG)

                def mtn():
                    rz = st["rz"]
                    st["m"] = gpool.tile([128, 2, BL], F32, tag=f"m{lt}",
                                         name="m")
                    nc.vector.tensor_tensor(st["m"], rz[:, 0:2, :],
                                            psAB[:, 4:6, :], MULT)
                    st["tn"] = gpool.tile([128, 2, BL], F32, tag=f"tn{lt}",
                                          name="tn")
                    nc.vector.tensor_tensor(st["tn"], st["m"],
                                            gxN[:, k, :, :], ADD)

                def uv():
                    rz = st["rz"]
                    st["u"] = gpool.tile([128, 2, BL], F32, tag=f"u{lt}",
                                         name="u")
                    nc.vector.tensor_tensor(st["u"], rz[:, 2:4, :], hsrc, MULT)
                    st["v"] = gpool.tile([128, 2, BL], F32, tag=f"v{lt}",
                                         name="v")
                    nc.vector.tensor_scalar(st["v"], rz[:, 2:4, :], -1.0, 1.0,
                                            MULT, ADD)

                def tanh():
                    st["n"] = gpool.tile([128, 2, BL], F32, tag=f"n{lt}",
                                         name="n")
                    nc.scalar.activation(st["n"], st["tn"], TANH)

                def upd():
                    st["w"] = gpool.tile([128, 2, BL], F32, tag=f"w{lt}",
                                         name="w")
                    nc.vector.tensor_tensor(st["w"], st["n"], st["v"], MULT)
                    nc.vector.tensor_tensor(hT_c[:, :, k, :], st["w"],
                                            st["u"], ADD)

                return pe, sig, mtn, uv, tanh, upd

            def step(k, hT_c, hT_p, gxA, gxN, whhT_t, psA_pool, rz_pool,
                     lt):
                for ph in step_phases(k, hT_c, hT_p, gxA, gxN, whhT_t,
                                      psA_pool, rz_pool, lt):
                    ph()

            def p5_out(u_prev, t0_prev):
                """out = y + h2 @ W_lin^T + b_lin for the previous super."""
                h2 = h2T[u_prev]
                for s in range(NSUB):
                    po = ps_misc.tile([64, I], F32, tag="ps", name="po")
                    for kc in range(2):
                        nc.tensor.matmul(po, h2[:, kc, ds(8 * s, 8), :],
                                         wlinT_t[:, kc, :],
                                         start=(kc == 0), stop=False)
                    nc.tensor.matmul(po, ones_bf[:, 0:64], blin_t,
                                     start=False, stop=True)
                    nc.vector.tensor_tensor(o_t[:, s, :], po,
                                            y_t[u_prev][:, s, :], ADD)
                for s in range(NSUB):
                    nc.sync.dma_start(
                        out=out_d[:, ds(t0_prev + 8 * s, 8), :]
                        .transpose([1, 0, 2]),
                        in_=o_t[:, s, :])

            def p2p4(p, u, sidx=None):
                """Interleaved recurrences: L0 for super p (parity u), L1 for
                super p-1 (parity 1-u)."""
                for k in range(SUP):
                    ph0 = step_phases(k, h1T[u], h1T[1 - u], gxA0[u],
                                      gxN0[u], whh0T_t, ps_a0, ps_rz0, "0")
                    if p >= 1:
                        ph1 = step_phases(k, h2T[1 - u], h2T[u],
                                          gxA1[1 - u], gxN1[1 - u],
                                          whh1T_t, ps_a1, ps_rz1, "1")
                        for f0, f1 in zip(ph0, ph1):
                            f0()
                            f1()
                    else:
                        for f0 in ph0:
                            f0()
                if debug and sidx is not None:
                    nc.sync.dma_start(out=dbg1_d[sidx], in_=h1T[u][:])
                    nc.sync.dma_start(out=dbg3_d[sidx], in_=gxA0[u][:])
                    nc.sync.dma_start(out=dbg4_d[sidx], in_=gxN0[u][:].bitcast(F32))
                    if p >= 1:
                        nc.sync.dma_start(out=dbg2_d[sidx - 1], in_=h2T[1 - u][:])

            def emit_position(p, u, t0, sidx=None):
                p0_input(u, t0)
                gx_bulk(u, w0T_t, [xT_t[u]], bcol0_t, gxA0, gxN0,
                        act_copy=True)
                p2p4(p, u, sidx)
                gx_bulk(u, w1T_t, [h1T[u][:, 0, :, :], h1T[u][:, 1, :, :]],
                        bcol1_t, gxA1, gxN1, act_copy=False)
                if p >= 1:
                    p5_out(1 - u, t0 - SUP)

            # ------------- schedule -------------
            emit_position(0, 0, 0, sidx=0 if debug else None)
            emit_position(1, 1, SUP, sidx=1 if debug else None)
            if nsup > 2:
                if debug:
                    for p in range(2, nsup):
                        emit_position(p, p % 2, p * SUP, sidx=p)
                else:
                    with tc.For_i(2 * SUP, T, 2 * SUP,
                                  staggered_reset=True) as iv:
                        emit_position(2, 0, iv)
                        tc.stage_boundary()
                        tc.stage_boundary()
                        emit_position(3, 1, iv + SUP)
                        tc.stage_boundary()
            # epilogue: L1 + out for the last super
            u_last = (nsup - 1) % 2
            for k in range(SUP):
                step(k, h2T[u_last], h2T[1 - u_last], gxA1[u_last],
                     gxN1[u_last], whh1T_t, ps_a1, ps_rz1, "1")
            if debug:
                nc.sync.dma_start(out=dbg2_d[nsup - 1], in_=h2T[u_last][:])
            p5_out(u_last, T - SUP)

    nc.compile()
    return nc


def prep_weights(W_ih0, W_hh0, b_ih0, b_hh0, W_ih1, W_hh1, b_ih1, b_hh1,
                 W_lin, b_lin):
    """Host-side weight reshaping for the transposed-state kernel."""
    f = np.float32
    bf = mybir.dt.np(mybir.dt.bfloat16)

    def chunkT(W, K):  # [G, K] -> [K-part, 2, NCH, 128] (kc chunks of 128)
        A = W.reshape(NCH, 128, K // 128, 128)  # [j, c, kc, p]
        return np.ascontiguousarray(A.transpose(3, 2, 0, 1)).astype(bf)

    def bhnB(bh):  # [128, SUP, 2, BL] broadcast of b_hh n-part
        v = bh[2 * H:].reshape(2, 128).T.astype(bf)  # [p, c]
        return np.ascontiguousarray(
            np.broadcast_to(v[:, None, :, None], (128, SUP, 2, BL)))

    w0T = np.ascontiguousarray(
        W_ih0.reshape(NCH, 128, I).transpose(2, 0, 1)).astype(f)  # [i, j, c]
    bc = lambda bi, bh: np.stack(
        [bi[128 * j:128 * (j + 1)] +
         (bh[128 * j:128 * (j + 1)] if j < 4 else 0.0)
         for j in range(NCH)], axis=1).astype(f)  # [128, NCH]
    return {
        "w0T": w0T,
        "whh0T": chunkT(W_hh0, H),
        "bcol0": bc(b_ih0, b_hh0),
        "bhnB0": bhnB(b_hh0),
        "w1T": chunkT(W_ih1, H),
        "whh1T": chunkT(W_hh1, H),
        "bcol1": bc(b_ih1, b_hh1),
        "bhnB1": bhnB(b_hh1),
        "wlinT": np.ascontiguousarray(
            W_lin.T.reshape(2, 128, I).transpose(1, 0, 2)).astype(bf),
        "blin": b_lin.reshape(1, I).astype(bf),
        "eye64": np.eye(64, dtype=f),
        "eye128": np.eye(128, dtype=bf),
    }


_NC_CACHE = {}


def kernel(z, y, W_ih0, W_hh0, b_ih0, b_hh0, W_ih1, W_hh1, b_ih1, b_hh1,
           W_lin, b_lin, _trace=False, _debug=False):
    """Full-input entry point: shards over 8 cores, returns full output."""
    from concourse.bass_utils import run_bass_kernel_spmd

    z = np.asarray(z, np.float32)
    y = np.asarray(y, np.float32)
    T = z.shape[1]
    key = (T, _debug)
    if key not in _NC_CACHE:
        _NC_CACHE[key] = build_nc(T=T, debug=_debug)
    nc = _NC_CACHE[key]

    wmaps = prep_weights(np.asarray(W_ih0), np.asarray(W_hh0),
                         np.asarray(b_ih0), np.asarray(b_hh0),
                         np.asarray(W_ih1), np.asarray(W_hh1),
                         np.asarray(b_ih1), np.asarray(b_hh1),
                         np.asarray(W_lin), np.asarray(b_lin))
    in_maps = []
    for c in range(NCORES):
        sl = slice(c * BL, (c + 1) * BL)
        m = {"z": np.ascontiguousarray(z[sl]),
             "y": np.ascontiguousarray(y[sl])}
        m.update(wmaps)
        in_maps.append(m)

    res = run_bass_kernel_spmd(nc, in_maps, list(range(NCORES)), trace=_trace)
    outs = [res.results[c]["out"] for c in range(NCORES)]
    full = np.concatenate(outs, axis=0).astype(np.float32)
    if _trace or _debug:
        return full, res
    return full
